# revision 23
# baseline (speedup 1.0000x reference)
"""Trainium2 Bass kernel: LSTM (B=128,T=512,I=256,H=512) + cummax/concat head.

Strategy: data-parallel over batch across 8 NeuronCores (16 rows/core),
LSTM weights replicated.  Per core:
  Phase A: xg[t,b,g] = x @ W_ih.T + b precomputed with full-M matmuls
           (x transposed on the fly via the DMA xbar), stored bf16 in DRAM.
  Phase B: 512 sequential steps.  Gates live in PSUM as [16 batch, 2048]:
           xg_t injected with an identity matmul (start=True), then 4
           accumulating matmuls h_chunk.T @ W_hh.T_chunk.  Gate columns are
           reordered (i|f|o|g) so one Sigmoid call covers i,f,o.  Cell state
           c stays fp32 on-chip; h is bf16 and is transposed back to the
           [h,partition] lhsT layout with 16x128 xbar DMA transposes.
           cummax folds into the step loop as a running tensor_max.
Outputs: concat (bf16, upcast to f32 on host), h_last (bf16), c_last (f32).
"""

import sys

for _p in ("/opt/trn_rl_repo", "/root/.axon_site/_ro/trn_rl_repo"):
    if _p not in sys.path:
        sys.path.insert(0, _p)

import numpy as np
import ml_dtypes

import concourse.bacc as bacc
import concourse.bass as bass
import concourse.mybir as mybir
import concourse.tile as tile
from concourse.bass_utils import run_bass_kernel_spmd

F32 = mybir.dt.float32
BF16 = mybir.dt.float16      # 16-bit compute dtype (fp16: enough mantissa)
AFT = mybir.ActivationFunctionType
BF = np.float16

B, T_FULL, I_DIM, H = 128, 512, 256, 512
NC_CORES = 8
BL = B // NC_CORES          # 16 batch rows per core
G4 = 4 * H                  # 2048 gate columns
TCH = 128                   # time-chunk for phase A

_GRAPH_CACHE = {}


def _build(T):
    """Build + compile the per-core Bass graph (same graph on all 8 cores)."""
    nc = bacc.Bacc("TRN2", target_bir_lowering=False, debug=False,
                   num_devices=NC_CORES)

    x_d = nc.dram_tensor("x", [BL, T, I_DIM], F32, kind="ExternalInput")
    h0_d = nc.dram_tensor("h0", [BL, H], F32, kind="ExternalInput")
    c0_d = nc.dram_tensor("c0", [BL, H], F32, kind="ExternalInput")
    wih_d = nc.dram_tensor("wih", [2, 128, G4], BF16, kind="ExternalInput")
    whh_d = nc.dram_tensor("whh", [4, 128, G4], BF16, kind="ExternalInput")
    bias_d = nc.dram_tensor("bias", [1, G4], BF16, kind="ExternalInput")
    id_d = nc.dram_tensor("ident", [16, 16], BF16, kind="ExternalInput")
    id128_d = nc.dram_tensor("ident128", [128, 128], BF16,
                             kind="ExternalInput")

    cat_d = nc.dram_tensor("concat", [BL, T, H], BF16, kind="ExternalOutput")
    hL_d = nc.dram_tensor("h_last", [BL, H], BF16, kind="ExternalOutput")
    cL_d = nc.dram_tensor("c_last", [BL, H], F32, kind="ExternalOutput")

    xg_d = nc.dram_tensor("xg_scratch", [T, BL, G4], BF16, kind="Internal")

    with tile.TileContext(nc) as tc:
        with (
            tc.tile_pool(name="const", bufs=1) as cpool,
            tc.tile_pool(name="state", bufs=1) as spool,
        ):
            # ---- resident constants ----
            wih_sb = cpool.tile([128, 2, G4], BF16)
            whh_sb = cpool.tile([128, 4, G4], BF16)
            bias_sb = cpool.tile([1, G4], BF16)
            id_sb = cpool.tile([16, 16], BF16)
            id128_sb = cpool.tile([128, 128], BF16)
            ones_sb = cpool.tile([1, 128], BF16)
            for k in range(2):
                nc.sync.dma_start(wih_sb[:, k, :], wih_d[k])
            for k in range(4):
                nc.sync.dma_start(whh_sb[:, k, :], whh_d[k])
            nc.sync.dma_start(bias_sb[:], bias_d[:])
            nc.sync.dma_start(id_sb[:], id_d[:])
            nc.sync.dma_start(id128_sb[:], id128_d[:])
            nc.vector.memset(ones_sb[:], 1.0)

            # ---- persistent state ----
            c_f32 = spool.tile([16, H], F32)
            nc.sync.dma_start(c_f32[:], c0_d[:])
            c_sb = spool.tile([16, H], BF16)
            nc.vector.tensor_copy(c_sb[:], c_f32[:])

            # ---- Phase A: xg = x @ W_ih.T + b  (store fp16 to DRAM) ----
            with (
                tc.tile_pool(name="pha", bufs=3) as pa,
                tc.tile_pool(name="pha_ps", bufs=2, space="PSUM") as pa_ps,
                tc.tile_pool(name="pha_pt", bufs=2, space="PSUM") as pa_pt,
            ):
                for bb in range(BL):
                    for tch in range(T // TCH):
                        t0 = tch * TCH
                        x_f = pa.tile([TCH, I_DIM], F32)
                        nc.gpsimd.dma_start(x_f[:], x_d[bb, t0:t0 + TCH, :])
                        x_bf = pa.tile([TCH, I_DIM], BF16)
                        nc.vector.tensor_copy(x_bf[:], x_f[:])
                        xt_ps = pa_pt.tile([128, 2, TCH], BF16)
                        for k in range(2):
                            nc.tensor.transpose(
                                xt_ps[:, k, :],
                                x_bf[:, k * 128:(k + 1) * 128], id128_sb[:])
                        xt = pa.tile([128, 2, TCH], BF16)
                        nc.scalar.copy(xt[:], xt_ps[:])
                        xg_sb = pa.tile([TCH, G4], BF16)
                        for half in range(2):
                            pg = pa_ps.tile([TCH, G4 // 2], F32)
                            for nn in range(2):
                                s = slice(half * 1024 + nn * 512,
                                          half * 1024 + (nn + 1) * 512)
                                sh = slice(nn * 512, (nn + 1) * 512)
                                nc.tensor.matmul(pg[:, sh], xt[:, 0, :],
                                                 wih_sb[:, 0, s],
                                                 start=True, stop=False)
                                nc.tensor.matmul(pg[:, sh], xt[:, 1, :],
                                                 wih_sb[:, 1, s],
                                                 start=False, stop=False)
                                nc.tensor.matmul(pg[:, sh], ones_sb[:],
                                                 bias_sb[:, s],
                                                 start=False, stop=True)
                            if half == 0:
                                nc.vector.tensor_copy(
                                    xg_sb[:, 0:1024], pg[:])
                            else:
                                nc.scalar.copy(xg_sb[:, 1024:2048], pg[:])
                        nc.gpsimd.dma_start(xg_d[t0:t0 + TCH, bb, :],
                                            xg_sb[:])

            # ---- Phase B: the 512-step recurrence ----
            with (
                tc.tile_pool(name="phb", bufs=3) as pb,
                tc.tile_pool(name="phb_xg", bufs=4) as pb_xg,
                tc.tile_pool(name="phb_hT", bufs=2) as pb_hT,
                tc.tile_pool(name="phb_if", bufs=2, space="PSUM") as pb_if,
                tc.tile_pool(name="phb_og", bufs=1, space="PSUM") as pb_og,
                tc.tile_pool(name="phb_pT", bufs=2, space="PSUM") as pb_pT,
            ):
                # init hT from h0, m from -inf
                h0_f = pb.tile([16, H], F32)
                nc.gpsimd.dma_start(h0_f[:], h0_d[:])
                h_bf = pb.tile([16, H], BF16, tag="h_bf")
                nc.vector.tensor_copy(h_bf[:], h0_f[:])
                psT = pb_pT.tile([128, 64], BF16, tag="psT")
                for kk in range(4):
                    nc.tensor.transpose(psT[:, kk * 16:(kk + 1) * 16],
                                        h_bf[:, kk * 128:(kk + 1) * 128],
                                        id_sb[:])
                hT = pb_hT.tile([128, 64], BF16, tag="hT")
                nc.vector.tensor_copy(hT[:], psT[:])
                m_sb = pb.tile([16, H // 2], BF16, tag="m_sb")
                nc.vector.memset(m_sb[:], -3.0e38)

                def load_xg(tt):
                    xgt = pb_xg.tile([16, G4], BF16, tag="xgt")
                    nc.gpsimd.dma_start(xgt[:], xg_d[tt])
                    return xgt

                def inject_if(xgt):
                    """Seed the next step's i|f PSUM (start=True MMs) —
                    software-pipelined so these fill the PE tail idle and
                    keep the HAM clock-gate warm."""
                    ifp = pb_if.tile([16, 2 * H], F32, tag="if_ps")
                    for nn in range(2):
                        s = slice(nn * 512, (nn + 1) * 512)
                        nc.tensor.matmul(ifp[:, s], id_sb[:], xgt[:, s],
                                         start=True, stop=False)
                    return ifp

                xg_cur = load_xg(0)
                if_ps = inject_if(xg_cur)
                for t in range(T):
                    if t + 1 < T:
                        xg_nxt = load_xg(t + 1)
                    # gate columns: i|f in if_ps, o|g in og_ps.
                    # MM order: f (critical c-path), i, then g, o.
                    sig = [None, None]
                    for nn in (1, 0):
                        s = slice(nn * 512, (nn + 1) * 512)
                        for kk in range(4):
                            nc.tensor.matmul(
                                if_ps[:, s], hT[:, kk * 16:(kk + 1) * 16],
                                whh_sb[:, kk, s],
                                start=False, stop=(kk == 3))
                        sig[nn] = pb.tile([16, H], BF16, tag=f"sig{nn}",
                                          name=f"sig{nn}")
                        nc.scalar.activation(sig[nn][:], if_ps[:, s],
                                             AFT.Sigmoid)
                    sig_i, sig_f = sig
                    og_ps = pb_og.tile([16, 2 * H], F32, tag="og_ps")
                    for nn in (1, 0):   # g first (ig path), o second
                        so = slice(nn * 512, (nn + 1) * 512)
                        sg = slice(2 * H + nn * 512, 2 * H + (nn + 1) * 512)
                        nc.tensor.matmul(og_ps[:, so], id_sb[:],
                                         xg_cur[:, sg], start=True,
                                         stop=False)
                        for kk in range(4):
                            nc.tensor.matmul(
                                og_ps[:, so], hT[:, kk * 16:(kk + 1) * 16],
                                whh_sb[:, kk, sg],
                                start=False, stop=(kk == 3))
                        if nn == 1:
                            gg = pb.tile([16, H], BF16, tag="gg")
                            nc.scalar.activation(gg[:], og_ps[:, H:2 * H],
                                                 AFT.Tanh)
                    sig_o = pb.tile([16, H], BF16, tag="sig_o")
                    sigo_ins = nc.scalar.activation(sig_o[:], og_ps[:, 0:H],
                                                    AFT.Sigmoid)
                    if t + 1 < T:
                        if_nxt = inject_if(xg_nxt)
                    nc.vector.tensor_mul(c_sb[:], sig_f[:], c_sb[:])
                    ig = pb.tile([16, H], BF16, tag="ig")
                    nc.vector.tensor_mul(ig[:], sig_i[:], gg[:])
                    nc.vector.tensor_add(c_sb[:], c_sb[:], ig[:])
                    tc_t = pb.tile([16, H], BF16, tag="tc_t")
                    tc_ins = nc.scalar.activation(tc_t[:], c_sb[:], AFT.Tanh)
                    # keep sig_o ahead of tanh_c in the ACT queue: h needs it
                    tile.add_dep_helper(tc_ins.ins, sigo_ins.ins, sync=False,
                                        reason="sig_o before tanh_c on ACT")
                    h_bf = pb.tile([16, H], BF16, tag="h_bf")
                    nc.vector.tensor_mul(h_bf[:], sig_o[:], tc_t[:])
                    psT = pb_pT.tile([128, 64], BF16, tag="psT")
                    for kk in range(4):
                        nc.tensor.transpose(psT[:, kk * 16:(kk + 1) * 16],
                                            h_bf[:, kk * 128:(kk + 1) * 128],
                                            id_sb[:])
                    hT = pb_hT.tile([128, 64], BF16, tag="hT")
                    nc.vector.tensor_copy(hT[:], psT[:])
                    m_prev, m_sb = m_sb, pb.tile([16, H // 2], BF16,
                                                 tag="m_sb")
                    nc.vector.tensor_max(m_sb[:], m_prev[:], h_bf[:, 0:H // 2])
                    nc.gpsimd.dma_start(cat_d[:, t, 0:H // 2], m_sb[:])
                    nc.gpsimd.dma_start(cat_d[:, t, H // 2:H],
                                        h_bf[:, H // 2:H])
                    if t + 1 < T:
                        xg_cur, if_ps = xg_nxt, if_nxt

                nc.gpsimd.dma_start(hL_d[:], h_bf[:])
                nc.vector.tensor_copy(c_f32[:], c_sb[:])
                nc.gpsimd.dma_start(cL_d[:], c_f32[:])

    nc.compile()
    return nc


def _get_graph(T):
    if T not in _GRAPH_CACHE:
        _GRAPH_CACHE[T] = _build(T)
    return _GRAPH_CACHE[T]


def _prep_weights(W_ih, W_hh, b):
    # pytorch gate order (i,f,g,o) -> (i,f,o,g) so sigmoid covers a prefix
    perm = np.r_[0:H, H:2 * H, 3 * H:4 * H, 2 * H:3 * H]
    wih = np.ascontiguousarray(
        W_ih[perm].T.reshape(2, 128, G4).astype(BF))
    whh = np.ascontiguousarray(
        W_hh[perm].T.reshape(4, 128, G4).astype(BF))
    bias = np.ascontiguousarray(b[perm].reshape(1, G4).astype(BF))
    ident = np.eye(16, dtype=BF)
    ident128 = np.eye(128, dtype=BF)
    return wih, whh, bias, ident, ident128


def kernel(input_tensor, h0, c0, W_ih, W_hh, b, _trace=False):
    input_tensor = np.asarray(input_tensor, dtype=np.float32)
    h0 = np.asarray(h0, dtype=np.float32)
    c0 = np.asarray(c0, dtype=np.float32)
    T = input_tensor.shape[1]

    wih, whh, bias, ident, ident128 = _prep_weights(
        np.asarray(W_ih, np.float32), np.asarray(W_hh, np.float32),
        np.asarray(b, np.float32))

    in_maps = []
    for c in range(NC_CORES):
        sl = slice(c * BL, (c + 1) * BL)
        in_maps.append({
            "x": np.ascontiguousarray(input_tensor[sl]),
            "h0": np.ascontiguousarray(h0[sl]),
            "c0": np.ascontiguousarray(c0[sl]),
            "wih": wih, "whh": whh, "bias": bias, "ident": ident,
            "ident128": ident128,
        })

    nc = _get_graph(T)
    res = run_bass_kernel_spmd(nc, in_maps, core_ids=list(range(NC_CORES)),
                               trace=_trace)
    kernel.last_exec_time_ns = res.exec_time_ns

    cat = np.concatenate([r["concat"].astype(np.float32) for r in res.results],
                         axis=0)
    hL = np.concatenate([r["h_last"].astype(np.float32) for r in res.results],
                        axis=0)
    cL = np.concatenate([r["c_last"] for r in res.results], axis=0)
    return cat, hL, cL


kernel.last_exec_time_ns = None


# revision 26
# speedup vs baseline: 1.0698x; 1.0698x over previous
"""Trainium2 Bass kernel: LSTM (B=128,T=512,I=256,H=512) + cummax/concat head.

Strategy: data-parallel over batch across 8 NeuronCores (16 rows/core),
LSTM weights replicated.  Per core:
  Phase A: xg[t,b,g] = x @ W_ih.T + b precomputed with full-M matmuls
           (x transposed on the fly via the DMA xbar), stored bf16 in DRAM.
  Phase B: 512 sequential steps.  Gates live in PSUM as [16 batch, 2048]:
           xg_t injected with an identity matmul (start=True), then 4
           accumulating matmuls h_chunk.T @ W_hh.T_chunk.  Gate columns are
           reordered (i|f|o|g) so one Sigmoid call covers i,f,o.  Cell state
           c stays fp32 on-chip; h is bf16 and is transposed back to the
           [h,partition] lhsT layout with 16x128 xbar DMA transposes.
           cummax folds into the step loop as a running tensor_max.
Outputs: concat (bf16, upcast to f32 on host), h_last (bf16), c_last (f32).
"""

import sys

for _p in ("/opt/trn_rl_repo", "/root/.axon_site/_ro/trn_rl_repo"):
    if _p not in sys.path:
        sys.path.insert(0, _p)

import numpy as np
import ml_dtypes

import concourse.bacc as bacc
import concourse.bass as bass
import concourse.mybir as mybir
import concourse.tile as tile
from concourse.bass_utils import run_bass_kernel_spmd

F32 = mybir.dt.float32
BF16 = mybir.dt.float16      # 16-bit compute dtype (fp16: enough mantissa)
AFT = mybir.ActivationFunctionType
BF = np.float16

B, T_FULL, I_DIM, H = 128, 512, 256, 512
NC_CORES = 8
BL = B // NC_CORES          # 16 batch rows per core
G4 = 4 * H                  # 2048 gate columns
TCH = 128                   # time-chunk for phase A

_GRAPH_CACHE = {}


def _build(T):
    """Build + compile the per-core Bass graph (same graph on all 8 cores)."""
    nc = bacc.Bacc("TRN2", target_bir_lowering=False, debug=False,
                   num_devices=NC_CORES)

    x_d = nc.dram_tensor("x", [BL, T, I_DIM], F32, kind="ExternalInput")
    h0_d = nc.dram_tensor("h0", [BL, H], F32, kind="ExternalInput")
    c0_d = nc.dram_tensor("c0", [BL, H], F32, kind="ExternalInput")
    wih_d = nc.dram_tensor("wih", [2, 128, G4], BF16, kind="ExternalInput")
    whh_d = nc.dram_tensor("whh", [4, 128, G4], BF16, kind="ExternalInput")
    bias_d = nc.dram_tensor("bias", [1, G4], BF16, kind="ExternalInput")
    id_d = nc.dram_tensor("ident", [16, 16], BF16, kind="ExternalInput")
    id128_d = nc.dram_tensor("ident128", [128, 128], BF16,
                             kind="ExternalInput")

    cat_d = nc.dram_tensor("concat", [BL, T, H], BF16, kind="ExternalOutput")
    hL_d = nc.dram_tensor("h_last", [BL, H], BF16, kind="ExternalOutput")
    cL_d = nc.dram_tensor("c_last", [BL, H], F32, kind="ExternalOutput")

    xg_d = nc.dram_tensor("xg_scratch", [T, BL, G4], BF16, kind="Internal")

    with tile.TileContext(nc) as tc:
        with (
            tc.tile_pool(name="const", bufs=1) as cpool,
            tc.tile_pool(name="state", bufs=1) as spool,
        ):
            # ---- resident constants ----
            wih_sb = cpool.tile([128, 2, G4], BF16)
            whh_sb = cpool.tile([128, 4, G4], BF16)
            bias_sb = cpool.tile([1, G4], BF16)
            id_sb = cpool.tile([16, 16], BF16)
            id128_sb = cpool.tile([128, 128], BF16)
            ones_sb = cpool.tile([1, 128], BF16)
            for k in range(2):
                nc.sync.dma_start(wih_sb[:, k, :], wih_d[k])
            for k in range(4):
                nc.sync.dma_start(whh_sb[:, k, :], whh_d[k])
            nc.sync.dma_start(bias_sb[:], bias_d[:])
            nc.sync.dma_start(id_sb[:], id_d[:])
            nc.sync.dma_start(id128_sb[:], id128_d[:])
            nc.vector.memset(ones_sb[:], 1.0)

            # ---- persistent state ----
            c_f32 = spool.tile([16, H], F32)
            nc.sync.dma_start(c_f32[:], c0_d[:])
            c_sb = spool.tile([16, H], BF16)
            nc.vector.tensor_copy(c_sb[:], c_f32[:])

            # ---- Phase A: xg = x @ W_ih.T + b  (store fp16 to DRAM) ----
            with (
                tc.tile_pool(name="pha", bufs=3) as pa,
                tc.tile_pool(name="pha_ps", bufs=2, space="PSUM") as pa_ps,
                tc.tile_pool(name="pha_pt", bufs=2, space="PSUM") as pa_pt,
            ):
                for bb in range(BL):
                    for tch in range(T // TCH):
                        t0 = tch * TCH
                        x_f = pa.tile([TCH, I_DIM], F32)
                        nc.gpsimd.dma_start(x_f[:], x_d[bb, t0:t0 + TCH, :])
                        x_bf = pa.tile([TCH, I_DIM], BF16)
                        nc.vector.tensor_copy(x_bf[:], x_f[:])
                        xt_ps = pa_pt.tile([128, 2, TCH], BF16)
                        for k in range(2):
                            nc.tensor.transpose(
                                xt_ps[:, k, :],
                                x_bf[:, k * 128:(k + 1) * 128], id128_sb[:])
                        xt = pa.tile([128, 2, TCH], BF16)
                        nc.scalar.copy(xt[:], xt_ps[:])
                        xg_sb = pa.tile([TCH, G4], BF16)
                        for half in range(2):
                            pg = pa_ps.tile([TCH, G4 // 2], F32)
                            for nn in range(2):
                                s = slice(half * 1024 + nn * 512,
                                          half * 1024 + (nn + 1) * 512)
                                sh = slice(nn * 512, (nn + 1) * 512)
                                nc.tensor.matmul(pg[:, sh], xt[:, 0, :],
                                                 wih_sb[:, 0, s],
                                                 start=True, stop=False)
                                nc.tensor.matmul(pg[:, sh], xt[:, 1, :],
                                                 wih_sb[:, 1, s],
                                                 start=False, stop=False)
                                nc.tensor.matmul(pg[:, sh], ones_sb[:],
                                                 bias_sb[:, s],
                                                 start=False, stop=True)
                            if half == 0:
                                nc.vector.tensor_copy(
                                    xg_sb[:, 0:1024], pg[:])
                            else:
                                nc.scalar.copy(xg_sb[:, 1024:2048], pg[:])
                        nc.gpsimd.dma_start(xg_d[t0:t0 + TCH, bb, :],
                                            xg_sb[:])

            # ---- Phase B: the 512-step recurrence ----
            with (
                tc.tile_pool(name="phb", bufs=3) as pb,
                tc.tile_pool(name="phb_xg", bufs=4) as pb_xg,
                tc.tile_pool(name="phb_hT", bufs=2) as pb_hT,
                tc.tile_pool(name="phb_if", bufs=2, space="PSUM") as pb_if,
                tc.tile_pool(name="phb_og", bufs=1, space="PSUM") as pb_og,
                tc.tile_pool(name="phb_pT", bufs=2, space="PSUM") as pb_pT,
            ):
                # init hT from h0, m from -inf
                h0_f = pb.tile([16, H], F32)
                nc.gpsimd.dma_start(h0_f[:], h0_d[:])
                h_bf = pb.tile([16, H], BF16, tag="h_bf")
                nc.vector.tensor_copy(h_bf[:], h0_f[:])
                psT = pb_pT.tile([128, 128], BF16, tag="psT")
                for kk in range(4):
                    nc.tensor.transpose(psT[:, kk * 16:(kk + 1) * 16],
                                        h_bf[:, kk * 128:(kk + 1) * 128],
                                        id_sb[:])
                hT = pb_hT.tile([128, 64], BF16, tag="hT")
                nc.vector.tensor_copy(hT[:], psT[:, 0:64])
                m_sb = pb.tile([16, H // 2], BF16, tag="m_sb")
                nc.vector.memset(m_sb[:], -3.0e38)

                def load_xg(tt):
                    xgt = pb_xg.tile([16, G4], BF16, tag="xgt")
                    nc.gpsimd.dma_start(xgt[:], xg_d[tt])
                    return xgt

                def inject_if(xgt):
                    """Seed the next step's i|f PSUM (start=True MMs) —
                    software-pipelined so these fill the PE tail idle and
                    keep the HAM clock-gate warm."""
                    ifp = pb_if.tile([16, 2 * H], F32, tag="if_ps")
                    for nn in range(2):
                        s = slice(nn * 512, (nn + 1) * 512)
                        nc.tensor.matmul(ifp[:, s], id_sb[:], xgt[:, s],
                                         start=True, stop=False)
                    return ifp

                xg_cur = load_xg(0)
                if_ps = inject_if(xg_cur)
                for t in range(T):
                    if t + 1 < T:
                        xg_nxt = load_xg(t + 1)
                    # i|f chain: 8 accumulating MMs, one merged sigmoid.
                    for nn in range(2):
                        s = slice(nn * 512, (nn + 1) * 512)
                        for kk in range(4):
                            nc.tensor.matmul(
                                if_ps[:, s], hT[:, kk * 16:(kk + 1) * 16],
                                whh_sb[:, kk, s],
                                start=False, stop=(kk == 3))
                    sig_if = pb.tile([16, 2 * H], BF16, tag="sig_if")
                    nc.scalar.activation(sig_if[:], if_ps[:], AFT.Sigmoid)
                    og_ps = pb_og.tile([16, 2 * H], F32, tag="og_ps")
                    for nn in (1, 0):   # g first (ig path), o second
                        so = slice(nn * 512, (nn + 1) * 512)
                        sg = slice(2 * H + nn * 512, 2 * H + (nn + 1) * 512)
                        nc.tensor.matmul(og_ps[:, so], id_sb[:],
                                         xg_cur[:, sg], start=True,
                                         stop=False)
                        for kk in range(4):
                            nc.tensor.matmul(
                                og_ps[:, so], hT[:, kk * 16:(kk + 1) * 16],
                                whh_sb[:, kk, sg],
                                start=False, stop=(kk == 3))
                        if nn == 1:
                            gg = pb.tile([16, H], BF16, tag="gg")
                            nc.scalar.activation(gg[:], og_ps[:, H:2 * H],
                                                 AFT.Tanh)
                    sig_o = pb.tile([16, H], BF16, tag="sig_o")
                    nc.scalar.activation(sig_o[:], og_ps[:, 0:H], AFT.Sigmoid)
                    if t + 1 < T:
                        if_nxt = inject_if(xg_nxt)
                    # transpose sig_o early (cols 64:128 of psT); tanh_c's
                    # transpose lands in cols 0:64; hT = oT * tcT so the
                    # batch-layout h multiply leaves the critical chain.
                    psT = pb_pT.tile([128, 128], BF16, tag="psT")
                    for kk in range(4):
                        nc.tensor.transpose(
                            psT[:, 64 + kk * 16:64 + (kk + 1) * 16],
                            sig_o[:, kk * 128:(kk + 1) * 128], id_sb[:])
                    oT = pb.tile([128, 64], BF16, tag="oT")
                    nc.vector.tensor_copy(oT[:], psT[:, 64:128])
                    nc.vector.tensor_mul(c_sb[:], sig_if[:, H:2 * H], c_sb[:])
                    ig = pb.tile([16, H], BF16, tag="ig")
                    nc.vector.tensor_mul(ig[:], sig_if[:, 0:H], gg[:])
                    nc.vector.tensor_add(c_sb[:], c_sb[:], ig[:])
                    tc_t = pb.tile([16, H], BF16, tag="tc_t")
                    nc.scalar.activation(tc_t[:], c_sb[:], AFT.Tanh)
                    for kk in range(4):
                        nc.tensor.transpose(psT[:, kk * 16:(kk + 1) * 16],
                                            tc_t[:, kk * 128:(kk + 1) * 128],
                                            id_sb[:])
                    hT = pb_hT.tile([128, 64], BF16, tag="hT")
                    nc.vector.tensor_mul(hT[:], oT[:], psT[:, 0:64])
                    # batch-layout h (output + cummax) — off the chain
                    h_bf = pb.tile([16, H], BF16, tag="h_bf")
                    nc.vector.tensor_mul(h_bf[:], sig_o[:], tc_t[:])
                    m_prev, m_sb = m_sb, pb.tile([16, H // 2], BF16,
                                                 tag="m_sb")
                    nc.vector.tensor_max(m_sb[:], m_prev[:], h_bf[:, 0:H // 2])
                    nc.gpsimd.dma_start(cat_d[:, t, 0:H // 2], m_sb[:])
                    nc.gpsimd.dma_start(cat_d[:, t, H // 2:H],
                                        h_bf[:, H // 2:H])
                    if t + 1 < T:
                        xg_cur, if_ps = xg_nxt, if_nxt

                nc.gpsimd.dma_start(hL_d[:], h_bf[:])
                nc.vector.tensor_copy(c_f32[:], c_sb[:])
                nc.gpsimd.dma_start(cL_d[:], c_f32[:])

    nc.compile()
    return nc


def _get_graph(T):
    if T not in _GRAPH_CACHE:
        _GRAPH_CACHE[T] = _build(T)
    return _GRAPH_CACHE[T]


def _prep_weights(W_ih, W_hh, b):
    # pytorch gate order (i,f,g,o) -> (i,f,o,g) so sigmoid covers a prefix
    perm = np.r_[0:H, H:2 * H, 3 * H:4 * H, 2 * H:3 * H]
    wih = np.ascontiguousarray(
        W_ih[perm].T.reshape(2, 128, G4).astype(BF))
    whh = np.ascontiguousarray(
        W_hh[perm].T.reshape(4, 128, G4).astype(BF))
    bias = np.ascontiguousarray(b[perm].reshape(1, G4).astype(BF))
    ident = np.eye(16, dtype=BF)
    ident128 = np.eye(128, dtype=BF)
    return wih, whh, bias, ident, ident128


def kernel(input_tensor, h0, c0, W_ih, W_hh, b, _trace=False):
    input_tensor = np.asarray(input_tensor, dtype=np.float32)
    h0 = np.asarray(h0, dtype=np.float32)
    c0 = np.asarray(c0, dtype=np.float32)
    T = input_tensor.shape[1]

    wih, whh, bias, ident, ident128 = _prep_weights(
        np.asarray(W_ih, np.float32), np.asarray(W_hh, np.float32),
        np.asarray(b, np.float32))

    in_maps = []
    for c in range(NC_CORES):
        sl = slice(c * BL, (c + 1) * BL)
        in_maps.append({
            "x": np.ascontiguousarray(input_tensor[sl]),
            "h0": np.ascontiguousarray(h0[sl]),
            "c0": np.ascontiguousarray(c0[sl]),
            "wih": wih, "whh": whh, "bias": bias, "ident": ident,
            "ident128": ident128,
        })

    nc = _get_graph(T)
    res = run_bass_kernel_spmd(nc, in_maps, core_ids=list(range(NC_CORES)),
                               trace=_trace)
    kernel.last_exec_time_ns = res.exec_time_ns

    cat = np.concatenate([r["concat"].astype(np.float32) for r in res.results],
                         axis=0)
    hL = np.concatenate([r["h_last"].astype(np.float32) for r in res.results],
                        axis=0)
    cL = np.concatenate([r["c_last"] for r in res.results], axis=0)
    return cat, hL, cL


kernel.last_exec_time_ns = None
